# revision 45
# baseline (speedup 1.0000x reference)
"""Trainium2 Bass kernel for a 3x3 VALID conv2d (dense_cnn).

reference: out[b,o,i,j] = sum_{c,kh,kw} x[b,c,i+kh,j+kw] * w[o,c,kh,kw]
  x: (32, 128, 64, 64) f32, w: (256, 128, 3, 3) f32 -> out: (32, 256, 62, 62) f32

Strategy:
  - Data-parallel over batch: 32 images / 8 cores = 4 images per core;
    weights replicated (pre-transposed on host to [C=128, kh*kw=9, O=256]).
  - Conv = 9 shifted matmuls accumulated in PSUM. Contraction dim C=128 sits
    on the SBUF partition axis.  For an 8-row group of output rows the moving
    operand is x_sb[:, i0+kh : i0+kh+8, kw : kw+62] (N = 8*62 = 496 <= 512)
    and the stationary operand is w_sb[:, kh*3+kw, oc*128:(oc+1)*128].
  - fp32 data is bitcast to float32r for the matmul (full-rate fp32 mode at
    moving free dim >= 256).
"""

import numpy as np

import concourse.bass as bass
import concourse.bacc as bacc
import concourse.mybir as mybir
import concourse.tile as tile

N_CORES = 8
B, C, H, W = 32, 128, 64, 64
O, KH, KW = 256, 3, 3
OH, OW = H - KH + 1, W - KW + 1  # 62, 62
B_LOC = B // N_CORES  # 4
ROWS_PER_GROUP = 8
F32 = mybir.dt.float32
F32R = mybir.dt.float32r

_CACHE: dict = {}


def _build_program() -> bass.Bass:
    nc = bacc.Bacc("TRN2", target_bir_lowering=False, debug=False)

    x_d = nc.dram_tensor("x", [B_LOC, C, H, W], F32R, kind="ExternalInput")
    w_d = nc.dram_tensor("wt", [C, KH * KW, O], F32R, kind="ExternalInput")
    o_d = nc.dram_tensor("out", [B_LOC, O, OH, OW], F32, kind="ExternalOutput")
    x_ap, w_ap, o_ap = x_d.ap(), w_d.ap(), o_d.ap()

    groups = [(i0, min(ROWS_PER_GROUP, OH - i0)) for i0 in range(0, OH, ROWS_PER_GROUP)]

    with tile.TileContext(nc) as tc:
        with (
            tc.tile_pool(name="wpool", bufs=1) as wpool,
            tc.tile_pool(name="xpool", bufs=3) as xpool,
            tc.tile_pool(name="opool", bufs=6) as opool,
            tc.tile_pool(name="warm", bufs=1) as warm,
            tc.tile_pool(name="pspool", bufs=7, space="PSUM") as pspool,
            tc.tile_pool(name="pswarm", bufs=1, space="PSUM") as pswarm,
        ):
            # --- PE clock warm-up: dummy matmuls on a zeroed tile keep the
            # PE busy while the first input DMAs stream in, so the real
            # matmuls run at the full (ramped) clock from the start.
            wz = warm.tile([C, 128], F32R)
            nc.vector.memset(wz.bitcast(F32), 0.0)
            psw = pswarm.tile([128, 128], F32)
            for _ in range(12):
                nc.tensor.matmul(
                    psw, lhsT=wz, rhs=wz, start=True, stop=True
                )

            # --- input loads: first-needed-first, alternating the two HWDGE
            # issuing engines (SP via nc.sync, ACT via nc.scalar) so
            # descriptor generation isn't serialized on one sequencer.
            w_sb = wpool.tile([C, KH * KW, O], F32R)
            x_sbs = []
            for b in range(B_LOC):
                x_sbs.append(xpool.tile([C, H, W], F32R, name="x_sb", tag="x_sb"))

            issue = 0

            def in_dma(out_ap_, in_ap_):
                nonlocal issue
                eng = nc.scalar if issue % 2 == 0 else nc.sync
                eng.dma_start(out=out_ap_, in_=in_ap_)
                issue += 1

            # k=0 weights and image-0 rows 0..15 first (the first row-group's
            # working set), then the remaining weights and image-0 rows.
            in_dma(w_sb[:, 0:1, 0:128], w_ap[:, 0:1, 0:128])
            # rows 0..9 in one chunk: exactly the first row-group's x needs
            in_dma(x_sbs[0][:, 0:10, :], x_ap[0, :, 0:10, :])
            in_dma(w_sb[:, 1:3, 0:128], w_ap[:, 1:3, 0:128])
            in_dma(w_sb[:, 3:6, 0:128], w_ap[:, 3:6, 0:128])
            in_dma(w_sb[:, 6:9, 0:128], w_ap[:, 6:9, 0:128])
            for r0 in range(10, H, 8):
                r1 = min(r0 + 8, H)
                in_dma(x_sbs[0][:, r0:r1, :], x_ap[0, :, r0:r1, :])
            # second output-channel half of the weights: only needed once the
            # oc=1 pass starts, well after the oc=0 groups are underway
            in_dma(w_sb[:, 0:3, 128:256], w_ap[:, 0:3, 128:256])
            in_dma(w_sb[:, 3:6, 128:256], w_ap[:, 3:6, 128:256])
            in_dma(w_sb[:, 6:9, 128:256], w_ap[:, 6:9, 128:256])

            for b in range(B_LOC):
                x_sb = x_sbs[b]
                if b + 1 < B_LOC:
                    # prefetch next image while computing this one
                    for r0 in range(0, H, 16):
                        in_dma(
                            x_sbs[b + 1][:, r0 : r0 + 16, :],
                            x_ap[b + 1, :, r0 : r0 + 16, :],
                        )

                for oc in range(O // 128):
                    for i0, rows in groups:
                        ps = pspool.tile([128, rows, OW], F32)
                        for k in range(KH * KW):
                            kh, kw = divmod(k, KW)
                            nc.tensor.matmul(
                                ps,
                                lhsT=w_sb[:, k, oc * 128 : (oc + 1) * 128],
                                rhs=x_sb[:, i0 + kh : i0 + kh + rows, kw : kw + OW],
                                start=(k == 0),
                                stop=(k == KH * KW - 1),
                            )
                        o_sb = opool.tile([128, rows, OW], F32)
                        nc.vector.tensor_copy(out=o_sb, in_=ps)
                        nc.sync.dma_start(
                            out=o_ap[b, oc * 128 : (oc + 1) * 128, i0 : i0 + rows, :],
                            in_=o_sb,
                        )
    nc.compile()
    return nc


def _get_executor():
    """Build the Bass program once and wrap it in a cached jitted SPMD
    executor (the multi-core path of bass2jax.run_bass_via_pjrt, but with the
    jit object reused across calls so repeated invocations skip recompile)."""
    if "exec" in _CACHE:
        return _CACHE["exec"]

    import jax
    from jax.sharding import Mesh, PartitionSpec
    from jax.experimental.shard_map import shard_map

    from concourse import bass2jax as b2j

    nc = _build_program()
    b2j.install_neuronx_cc_hook()

    partition_name = nc.partition_id_tensor.name if nc.partition_id_tensor else None
    in_names: list[str] = []
    out_names: list[str] = []
    out_avals = []
    for alloc in nc.m.functions[0].allocations:
        if not isinstance(alloc, mybir.MemoryLocationSet):
            continue
        name = alloc.memorylocations[0].name
        if alloc.kind == "ExternalInput":
            if name != partition_name:
                in_names.append(name)
        elif alloc.kind == "ExternalOutput":
            shape = tuple(alloc.tensor_shape)
            dtype = mybir.dt.np(alloc.dtype)
            out_names.append(name)
            out_avals.append(jax.core.ShapedArray(shape, dtype))
    n_params = len(in_names)
    n_outs = len(out_avals)
    all_in_names = in_names + out_names
    if partition_name is not None:
        all_in_names.append(partition_name)
    donate = tuple(range(n_params, n_params + n_outs))

    def _body(*args):
        operands = list(args)
        if partition_name is not None:
            operands.append(b2j.partition_id_tensor())
        outs = b2j._bass_exec_p.bind(
            *operands,
            out_avals=tuple(out_avals),
            in_names=tuple(all_in_names),
            out_names=tuple(out_names),
            lowering_input_output_aliases=(),
            sim_require_finite=True,
            sim_require_nnan=True,
            nc=nc,
        )
        return tuple(outs)

    devices = jax.devices()[:N_CORES]
    mesh = Mesh(np.asarray(devices), ("core",))
    in_specs = (PartitionSpec("core"),) * (n_params + n_outs)
    out_specs = (PartitionSpec("core"),) * n_outs
    sharded = jax.jit(
        shard_map(_body, mesh=mesh, in_specs=in_specs, out_specs=out_specs,
                  check_rep=False),
        donate_argnums=donate,
        keep_unused=True,
    )

    zero_out_shapes = [
        ((N_CORES * a.shape[0], *a.shape[1:]), a.dtype) for a in out_avals
    ]

    def run(in_maps: list[dict[str, np.ndarray]]) -> list[dict[str, np.ndarray]]:
        concat_in = [
            np.concatenate([np.asarray(m[name]) for m in in_maps], axis=0)
            for name in in_names
        ]
        concat_zeros = [np.zeros(s, d) for s, d in zero_out_shapes]
        out_arrs = sharded(*concat_in, *concat_zeros)
        return [
            {
                name: np.asarray(out_arrs[i]).reshape(N_CORES, *out_avals[i].shape)[c]
                for i, name in enumerate(out_names)
            }
            for c in range(N_CORES)
        ]

    _CACHE["exec"] = run
    _CACHE["nc"] = nc
    return run


def kernel(x: np.ndarray, weights: np.ndarray) -> np.ndarray:
    x = np.ascontiguousarray(x, dtype=np.float32)
    # [o, c, kh, kw] -> [c, kh, kw, o] -> [c, kh*kw, o]
    wt = np.ascontiguousarray(
        np.asarray(weights, dtype=np.float32).transpose(1, 2, 3, 0).reshape(C, KH * KW, O)
    )
    run = _get_executor()
    in_maps = [
        {"x": x[i * B_LOC : (i + 1) * B_LOC], "wt": wt} for i in range(N_CORES)
    ]
    results = run(in_maps)
    return np.concatenate([r["out"] for r in results], axis=0)
